# revision 8
# baseline (speedup 1.0000x reference)
"""ContrastiveGCN forward on 8 TRN2 NeuronCores (Bass/Tile).

Structure (per higher-order GCN layer, max_order=2):
    out = A_hat(x@W_1 + A_hat(x@W_2)) + b            [A_hat commutes with @W]
with A_hat = dinv * (A+I) * dinv applied as: scale -> plain-sum aggregate -> scale.

Distribution: nodes sharded over 8 cores (6250 each), edges partitioned by dst
shard; every aggregation output shard is AllGathered into a full f32 table in
each core's DRAM which the next propagate gathers from (dma_gather, 256B rows).
Aggregation = per-window (64 dst slots) one-hot matmul: for each 128-edge tile,
PSUM[feat, slot] += msgs[edge, feat].T @ P[edge, slot].  All cores run one SPMD
program: per-(window, src-half) tile counts are padded to the max over cores.
"""
import numpy as np

N = 50000
E = 800000
IN_CH = 128
HID = 64
HID2 = 32
PROJ = 32
NC = 8
S = N // NC            # 6250 nodes per core
W_SLOTS = 64           # dst slots per window
NWIN = (S + W_SLOTS - 1) // W_SLOTS   # 98
HALF = N // 2          # src-half split for int16 gather indices
TILE_E = 128           # edges per tile
CALL_TILES = 32        # tiles per dma_gather / P-chunk


def _build_meta(src, dst):
    """Per-core edge metadata with core-homogenized tile counts.

    Returns dict with per-core idx streams (wrapped int16, per src-half),
    P streams (f32 one-hot [128, Tg*64]), and the shared program structure:
    for each window w and half h, the number of tiles (same for all cores).
    """
    core = dst // S
    meta = {"cores": []}
    # bucket edges per (core, window, half)
    buckets = [[None] * (NWIN * 2) for _ in range(NC)]
    for c in range(NC):
        m = core == c
        sc, dc = src[m], dst[m]
        dl = dc - c * S
        w = dl // W_SLOTS
        h = (sc >= HALF).astype(np.int64)
        key = w * 2 + h
        order = np.argsort(key, kind="stable")
        sc, dl, key = sc[order], dl[order], key[order]
        cnt = np.bincount(key, minlength=NWIN * 2)
        off = np.concatenate([[0], np.cumsum(cnt)])
        for k in range(NWIN * 2):
            buckets[c][k] = (sc[off[k]:off[k + 1]], dl[off[k]:off[k + 1]])
    # homogenized tile counts
    ntiles = np.zeros((NWIN, 2), np.int64)
    for w in range(NWIN):
        for h in range(2):
            mx = max(len(buckets[c][w * 2 + h][0]) for c in range(NC))
            ntiles[w, h] = max(1, (mx + TILE_E - 1) // TILE_E)
    nt_h = [int(ntiles[:, h].sum()) for h in range(2)]
    Tg = int(ntiles.sum())
    # stream positions
    half_pos = np.zeros((NWIN, 2), np.int64)   # first tile index within half-stream
    glob_pos = np.zeros((NWIN, 2), np.int64)   # first tile index within global P stream
    acc = [0, 0]
    g = 0
    for w in range(NWIN):
        for h in range(2):
            half_pos[w, h] = acc[h]
            glob_pos[w, h] = g
            acc[h] += ntiles[w, h]
            g += ntiles[w, h]
    meta.update(ntiles=ntiles, nt_h=nt_h, Tg=Tg, half_pos=half_pos, glob_pos=glob_pos)

    for c in range(NC):
        idx_h = []
        P = np.zeros((TILE_E, Tg * W_SLOTS), np.float32)
        for h in range(2):
            flat = np.zeros(nt_h[h] * TILE_E, np.int16)
            for w in range(NWIN):
                sc, dl = buckets[c][w * 2 + h]
                base_t = half_pos[w, h]
                ne = len(sc)
                loc = (sc - h * HALF).astype(np.int16)
                flat[base_t * TILE_E: base_t * TILE_E + ne] = loc
                # P entries: edge i of bucket -> tile base_g + i//128, row i%128,
                # col dl - w*W_SLOTS
                gi = glob_pos[w, h] + np.arange(ne) // TILE_E
                P[np.arange(ne) % TILE_E, gi * W_SLOTS + (dl - w * W_SLOTS)] = 1.0
            # wrap per call: position i -> [i%16, callcol0 + i//16]
            wrapped = np.zeros((128, nt_h[h] * 8), np.int16)
            t0 = 0
            while t0 < nt_h[h]:
                nt = min(CALL_TILES, nt_h[h] - t0)
                seg = flat[t0 * TILE_E:(t0 + nt) * TILE_E].reshape(nt * 8, 16).T
                for r in range(8):
                    wrapped[r * 16:(r + 1) * 16, t0 * 8:t0 * 8 + nt * 8] = seg
                t0 += nt
            idx_h.append(wrapped)
        meta["cores"].append({"idxA": idx_h[0], "idxB": idx_h[1], "P": P})
    return meta


def _build_program(meta):
    import concourse.bass as bass
    import concourse.bacc as bacc
    import concourse.mybir as mybir
    import concourse.tile as tile
    from concourse.masks import make_identity

    ntiles, half_pos, glob_pos = meta["ntiles"], meta["half_pos"], meta["glob_pos"]
    nt_h, Tg = meta["nt_h"], meta["Tg"]
    dt = mybir.dt
    F32 = dt.float32

    nc = bacc.Bacc("TRN2", target_bir_lowering=False, debug=False, num_devices=NC)
    # inputs
    xT = nc.dram_tensor("xT", [IN_CH, S], F32, kind="ExternalInput")
    dinv_in = nc.dram_tensor("dinv", [1, S], F32, kind="ExternalInput")
    idxA = nc.dram_tensor("idxA", [128, nt_h[0] * 8], dt.int16, kind="ExternalInput")
    idxB = nc.dram_tensor("idxB", [128, nt_h[1] * 8], dt.int16, kind="ExternalInput")
    Pst = nc.dram_tensor("Pst", [128, Tg * W_SLOTS], F32, kind="ExternalInput")
    wts = {}
    for nm, shp in [("W1_1", [IN_CH, HID]), ("W1_2", [IN_CH, HID]),
                    ("W2_1", [HID, HID2]), ("W2_2", [HID, HID2]),
                    ("Wp1", [HID2, PROJ]), ("Wp2", [PROJ, PROJ]),
                    ("Wc1", [HID2, 16]), ("Wc2", [16, 2]),
                    ("b1", [HID, 1]), ("b2", [HID2, 1]), ("bp1", [PROJ, 1]),
                    ("bp2", [PROJ, 1]), ("bc1", [16, 1]), ("bc2", [2, 1])]:
        wts[nm] = nc.dram_tensor(nm, shp, F32, kind="ExternalInput")
    zT_out = nc.dram_tensor("zT", [PROJ, S], F32, kind="ExternalOutput")
    cT_out = nc.dram_tensor("cT", [2, S], F32, kind="ExternalOutput")

    with tile.TileContext(nc) as tc:
        with (
            tc.tile_pool(name="persist", bufs=1) as pp,
            tc.tile_pool(name="gpool", bufs=2) as gp,
            tc.tile_pool(name="ppool", bufs=2) as plp,
            tc.tile_pool(name="wpool", bufs=3) as wp,
            tc.tile_pool(name="psum", bufs=4, space="PSUM") as psp,
            tc.tile_pool(name="psum2", bufs=2, space="PSUM") as psp2,
            tc.tile_pool(name="dram", bufs=1, space="DRAM") as dr,
        ):
            # --- static tiles
            ident = pp.tile([128, 128], F32)
            make_identity(nc, ident[:])
            ones1 = pp.tile([1, HID], F32)
            nc.vector.memset(ones1[:], 1.0)
            zeros_sb = pp.tile([64, 64], F32)
            nc.vector.memset(zeros_sb[:], 0.0)
            wsb = {}
            for nm in ["W1_1", "W1_2", "W2_1", "W2_2", "Wp1", "Wp2", "Wc1", "Wc2",
                       "b1", "b2", "bp1", "bp2", "bc1", "bc2"]:
                t = pp.tile(list(wts[nm].shape), F32, tag=nm)
                nc.sync.dma_start(t[:], wts[nm][:])
                wsb[nm] = t
            idxA_sb = pp.tile([128, nt_h[0] * 8], dt.int16)
            nc.sync.dma_start(idxA_sb[:], idxA[:])
            idxB_sb = pp.tile([128, nt_h[1] * 8], dt.int16)
            nc.sync.dma_start(idxB_sb[:], idxB[:])
            idx_sb = [idxA_sb, idxB_sb]

            dinvrep = pp.tile([64, S], F32)
            for c0 in range(0, S, 512):
                n = min(512, S - c0)
                dch = wp.tile([1, 512], F32, tag="tmp")
                nc.sync.dma_start(dch[:, :n], dinv_in[:, c0:c0 + n])
                ps = psp2.tile([64, 512], F32, tag="dps")
                nc.tensor.matmul(ps[:, :n], ones1[:], dch[:, :n],
                                 start=True, stop=True)
                nc.vector.tensor_copy(dinvrep[:, c0:c0 + n], ps[:, :n])

            # persistent feat-major activations
            a1p = pp.tile([HID, S], F32)     # dinv * (x@W1_1)
            h1T = pp.tile([HID, S], F32)
            a2p = pp.tile([HID2, S], F32)    # dinv * (h1@W2_1)
            h2T = pp.tile([HID2, S], F32)

            # tables + bounces (internal DRAM)
            tables = [dr.tile([N, HID], F32, name=f"table{i}", tag=f"table{i}",
                              addr_space="Shared") for i in range(4)]
            bounces = [dr.tile([S, HID], F32, name=f"bounce{i}", tag=f"bounce{i}")
                       for i in range(4)]

            def write_table_window(feat_sb, w, ncols, bounce, nrows_feat):
                """feat_sb: SBUF [64, ncols] feat-major window (rows 0:nrows_feat
                valid). Transpose -> node-major [ncols, 64] -> DMA bounce rows."""
                pst = psp2.tile([64, 64], F32, tag="tps")
                nc.tensor.transpose(pst[:ncols, :nrows_feat],
                                    feat_sb[:nrows_feat, :ncols],
                                    ident[:nrows_feat, :nrows_feat])
                ob = wp.tile([64, 64], F32, tag="ob")
                nc.vector.tensor_copy(ob[:ncols, :nrows_feat], pst[:ncols, :nrows_feat])
                if nrows_feat < HID:
                    nc.vector.tensor_copy(ob[:ncols, nrows_feat:HID],
                                          zeros_sb[:ncols, :HID - nrows_feat])
                nc.scalar.dma_start(bounce[w * W_SLOTS:w * W_SLOTS + ncols, :], ob[:ncols, :])

            def dense_chunks(lhsT_sb, rhs_sb, nrows_in, nrows_out, post):
                """out[r, node] = lhsT.T @ rhs per 512-col chunk; post(ps, c0, n)."""
                for c0 in range(0, S, 512):
                    n = min(512, S - c0)
                    ps = psp2.tile([nrows_out, 512], F32, tag="dps")
                    nc.tensor.matmul(ps[:, :n], lhsT_sb[:nrows_in, :],
                                     rhs_sb[:nrows_in, c0:c0 + n], start=True, stop=True)
                    post(ps, c0, n)

            # ---------- x-stage: t1 = dinv*(x@W1_2) -> bounce0 ; a1p = dinv*(x@W1_1)
            def post_t1(ps, c0, n):
                tmp = wp.tile([64, 512], F32, tag="tmp")
                nc.vector.tensor_tensor(tmp[:, :n], ps[:, :n],
                                        dinvrep[:, c0:c0 + n], op=mybir.AluOpType.mult)
                for w0 in range(0, n, W_SLOTS):
                    w = (c0 + w0) // W_SLOTS
                    ncols = min(W_SLOTS, n - w0)
                    write_table_window(tmp[:, w0:w0 + ncols], w, ncols, bounces[0], HID)

            def post_a1(ps, c0, n):
                nc.vector.tensor_tensor(a1p[:, c0:c0 + n], ps[:, :n],
                                        dinvrep[:, c0:c0 + n], op=mybir.AluOpType.mult)

            for c0 in range(0, S, 512):
                n = min(512, S - c0)
                xch = wp.tile([IN_CH, 512], F32, tag="tmp")
                nc.sync.dma_start(xch[:, :n], xT[:, c0:c0 + n])
                ps_u = psp2.tile([HID, 512], F32, tag="dps")
                nc.tensor.matmul(ps_u[:, :n], wsb["W1_2"][:], xch[:, :n],
                                 start=True, stop=True)
                post_t1(ps_u, c0, n)
                ps_a = psp2.tile([HID, 512], F32, tag="dps")
                nc.tensor.matmul(ps_a[:, :n], wsb["W1_1"][:], xch[:, :n],
                                 start=True, stop=True)
                post_a1(ps_a, c0, n)

            def allgather(i):
                nc.gpsimd.collective_compute(
                    "AllGather", mybir.AluOpType.bypass,
                    replica_groups=[list(range(NC))],
                    ins=[bounces[i].opt()], outs=[tables[i].opt()])

            allgather(0)

            # ---------- one propagate pass
            def propagate(table, nfeat, post_window):
                halves = [table[:HALF, :], table[HALF:, :]]
                gbufs = [[], []]
                ncalls = [(nt_h[h] + CALL_TILES - 1) // CALL_TILES for h in range(2)]
                for ci in range(max(ncalls)):
                    for h in range(2):
                        if ci >= ncalls[h]:
                            continue
                        t0 = ci * CALL_TILES
                        nt = min(CALL_TILES, nt_h[h] - t0)
                        gb = gp.tile([128, CALL_TILES * HID], F32, tag=f"gbuf{h}")
                        nc.gpsimd.dma_gather(
                            gb[:, :nt * HID].rearrange("p (k d) -> p k d", d=HID),
                            halves[h], idx_sb[h][:, t0 * 8:t0 * 8 + nt * 8],
                            nt * TILE_E, nt * TILE_E, HID, single_packet=False)
                        gbufs[h].append(gb)
                pbufs = []
                t0 = 0
                while t0 < Tg:
                    nt = min(CALL_TILES, Tg - t0)
                    pb = plp.tile([128, CALL_TILES * W_SLOTS], F32, tag="pbuf")
                    nc.sync.dma_start(pb[:, :nt * W_SLOTS],
                                      Pst[:, t0 * W_SLOTS:(t0 + nt) * W_SLOTS])
                    pbufs.append(pb)
                    t0 += CALL_TILES
                for w in range(NWIN):
                    ps = psp.tile([64, W_SLOTS], F32, tag="wps")
                    k = 0
                    ktot = int(ntiles[w, 0] + ntiles[w, 1])
                    for h in range(2):
                        for j in range(half_pos[w, h], half_pos[w, h] + ntiles[w, h]):
                            g = glob_pos[w, h] + (j - half_pos[w, h])
                            lhsT = gbufs[h][j // CALL_TILES][
                                :, (j % CALL_TILES) * HID:(j % CALL_TILES) * HID + nfeat]
                            rhs = pbufs[g // CALL_TILES][
                                :, (g % CALL_TILES) * W_SLOTS:(g % CALL_TILES + 1) * W_SLOTS]
                            nc.tensor.matmul(ps[:nfeat, :], lhsT, rhs,
                                             start=(k == 0), stop=(k == ktot - 1))
                            k += 1
                    ncols = min(W_SLOTS, S - w * W_SLOTS)
                    post_window(ps, w, ncols)

            # ---- P1a: v1 -> t2 = dinv2*v1raw + a1p -> bounce1
            def post_1a(ps, w, ncols):
                c0 = w * W_SLOTS
                tmp = wp.tile([64, W_SLOTS], F32, tag="tmp")
                nc.vector.tensor_tensor(tmp[:, :ncols], ps[:, :ncols],
                                        dinvrep[:, c0:c0 + ncols], op=mybir.AluOpType.mult)
                nc.vector.tensor_tensor(tmp[:, :ncols], tmp[:, :ncols],
                                        dinvrep[:, c0:c0 + ncols], op=mybir.AluOpType.mult)
                nc.vector.tensor_tensor(tmp[:, :ncols], tmp[:, :ncols],
                                        a1p[:, c0:c0 + ncols], op=mybir.AluOpType.add)
                write_table_window(tmp, w, ncols, bounces[1], HID)

            propagate(tables[0], HID, post_1a)
            allgather(1)

            # ---- P1b: h1 = relu(dinv*w1raw + b1) (feat-major, kept in SBUF)
            def post_1b(ps, w, ncols):
                c0 = w * W_SLOTS
                nc.vector.tensor_tensor(h1T[:, c0:c0 + ncols], ps[:, :ncols],
                                        dinvrep[:, c0:c0 + ncols], op=mybir.AluOpType.mult)
                nc.vector.tensor_tensor(h1T[:, c0:c0 + ncols], h1T[:, c0:c0 + ncols],
                                        wsb["b1"][:, :1].to_broadcast([HID, ncols]),
                                        op=mybir.AluOpType.add)
                nc.vector.tensor_scalar_max(h1T[:, c0:c0 + ncols],
                                            h1T[:, c0:c0 + ncols], 0.0)

            propagate(tables[1], HID, post_1b)

            # dense: t3 = dinv*(h1@W2_2) -> bounce2 ; a2p = dinv*(h1@W2_1)
            def post_t3(ps, c0, n):
                tmp = wp.tile([HID2, 512], F32, tag="tmp")
                nc.vector.tensor_tensor(tmp[:, :n], ps[:, :n],
                                        dinvrep[:HID2, c0:c0 + n], op=mybir.AluOpType.mult)
                for w0 in range(0, n, W_SLOTS):
                    w = (c0 + w0) // W_SLOTS
                    ncols = min(W_SLOTS, n - w0)
                    write_table_window(tmp[:, w0:w0 + ncols], w, ncols, bounces[2], HID2)

            def post_a2(ps, c0, n):
                nc.vector.tensor_tensor(a2p[:, c0:c0 + n], ps[:, :n],
                                        dinvrep[:HID2, c0:c0 + n], op=mybir.AluOpType.mult)

            dense_chunks(wsb["W2_2"], h1T, HID, HID2, post_t3)
            dense_chunks(wsb["W2_1"], h1T, HID, HID2, post_a2)
            allgather(2)

            # ---- P2a: t4 = dinv2*v2raw + a2p -> bounce3
            def post_2a(ps, w, ncols):
                c0 = w * W_SLOTS
                tmp = wp.tile([HID2, W_SLOTS], F32, tag="tmp")
                nc.vector.tensor_tensor(tmp[:, :ncols], ps[:HID2, :ncols],
                                        dinvrep[:HID2, c0:c0 + ncols], op=mybir.AluOpType.mult)
                nc.vector.tensor_tensor(tmp[:, :ncols], tmp[:, :ncols],
                                        dinvrep[:HID2, c0:c0 + ncols], op=mybir.AluOpType.mult)
                nc.vector.tensor_tensor(tmp[:, :ncols], tmp[:, :ncols],
                                        a2p[:, c0:c0 + ncols], op=mybir.AluOpType.add)
                write_table_window(tmp, w, ncols, bounces[3], HID2)

            propagate(tables[2], HID2, post_2a)
            allgather(3)

            # ---- P2b: h2 = relu(dinv*w2raw + b2)
            def post_2b(ps, w, ncols):
                c0 = w * W_SLOTS
                nc.vector.tensor_tensor(h2T[:, c0:c0 + ncols], ps[:HID2, :ncols],
                                        dinvrep[:HID2, c0:c0 + ncols], op=mybir.AluOpType.mult)
                nc.vector.tensor_tensor(h2T[:, c0:c0 + ncols], h2T[:, c0:c0 + ncols],
                                        wsb["b2"][:, :1].to_broadcast([HID2, ncols]),
                                        op=mybir.AluOpType.add)
                nc.vector.tensor_scalar_max(h2T[:, c0:c0 + ncols],
                                            h2T[:, c0:c0 + ncols], 0.0)

            propagate(tables[3], HID2, post_2b)

            # ---- heads
            for c0 in range(0, S, 512):
                n = min(512, S - c0)
                # projection head
                ps1 = psp2.tile([PROJ, 512], F32, tag="dps")
                nc.tensor.matmul(ps1[:, :n], wsb["Wp1"][:], h2T[:, c0:c0 + n],
                                 start=True, stop=True)
                ztmp = wp.tile([PROJ, 512], F32, tag="tmp")
                nc.vector.tensor_tensor(ztmp[:, :n], ps1[:, :n],
                                        wsb["bp1"][:, :1].to_broadcast([PROJ, n]),
                                        op=mybir.AluOpType.add)
                nc.vector.tensor_scalar_max(ztmp[:, :n], ztmp[:, :n], 0.0)
                ps2 = psp2.tile([PROJ, 512], F32, tag="dps")
                nc.tensor.matmul(ps2[:, :n], wsb["Wp2"][:], ztmp[:, :n],
                                 start=True, stop=True)
                zo = wp.tile([PROJ, 512], F32, tag="tmp")
                nc.vector.tensor_tensor(zo[:, :n], ps2[:, :n],
                                        wsb["bp2"][:, :1].to_broadcast([PROJ, n]),
                                        op=mybir.AluOpType.add)
                nc.scalar.dma_start(zT_out[:, c0:c0 + n], zo[:, :n])
                # coordinate head
                ps3 = psp2.tile([16, 512], F32, tag="dps")
                nc.tensor.matmul(ps3[:, :n], wsb["Wc1"][:], h2T[:, c0:c0 + n],
                                 start=True, stop=True)
                ctmp = wp.tile([16, 512], F32, tag="tmp")
                nc.vector.tensor_tensor(ctmp[:, :n], ps3[:, :n],
                                        wsb["bc1"][:, :1].to_broadcast([16, n]),
                                        op=mybir.AluOpType.add)
                nc.vector.tensor_scalar_max(ctmp[:, :n], ctmp[:, :n], 0.0)
                ps4 = psp2.tile([2, 512], F32, tag="dps")
                nc.tensor.matmul(ps4[:, :n], wsb["Wc2"][:], ctmp[:, :n],
                                 start=True, stop=True)
                co = wp.tile([2, 512], F32, tag="tmp")
                nc.vector.tensor_tensor(co[:, :n], ps4[:, :n],
                                        wsb["bc2"][:, :1].to_broadcast([2, n]),
                                        op=mybir.AluOpType.add)
                nc.scalar.dma_start(cT_out[:, c0:c0 + n], co[:, :n])

    nc.compile()
    return nc


_CACHE = {}


def _get_compiled(src, dst):
    key = "prog"
    if key not in _CACHE:
        meta = _build_meta(src, dst)
        nc = _build_program(meta)
        _CACHE[key] = (meta, nc)
    return _CACHE[key]


def kernel(x, edge_index, W1_1, W1_2, b1, W2_1, W2_2, b2,
           Wp1, bp1, Wp2, bp2, Wc1, bc1, Wc2, bc2):
    from concourse import bass_utils

    x = np.asarray(x, np.float32)
    ei = np.asarray(edge_index)
    loop = np.arange(N, dtype=np.int64)
    src = np.concatenate([ei[0], loop])
    dst = np.concatenate([ei[1], loop])
    deg = np.bincount(dst, minlength=N).astype(np.float32)
    dinv = deg ** -0.5

    meta, nc = _get_compiled(src, dst)

    xT = np.ascontiguousarray(x.T)
    in_maps = []
    for c in range(NC):
        m = meta["cores"][c]
        im = {
            "xT": np.ascontiguousarray(xT[:, c * S:(c + 1) * S]),
            "dinv": np.ascontiguousarray(dinv[None, c * S:(c + 1) * S]),
            "idxA": m["idxA"], "idxB": m["idxB"], "Pst": m["P"],
            "W1_1": np.asarray(W1_1, np.float32), "W1_2": np.asarray(W1_2, np.float32),
            "W2_1": np.asarray(W2_1, np.float32), "W2_2": np.asarray(W2_2, np.float32),
            "Wp1": np.asarray(Wp1, np.float32), "Wp2": np.asarray(Wp2, np.float32),
            "Wc1": np.asarray(Wc1, np.float32), "Wc2": np.asarray(Wc2, np.float32),
            "b1": np.asarray(b1, np.float32).reshape(HID, 1),
            "b2": np.asarray(b2, np.float32).reshape(HID2, 1),
            "bp1": np.asarray(bp1, np.float32).reshape(PROJ, 1),
            "bp2": np.asarray(bp2, np.float32).reshape(PROJ, 1),
            "bc1": np.asarray(bc1, np.float32).reshape(16, 1),
            "bc2": np.asarray(bc2, np.float32).reshape(2, 1),
        }
        in_maps.append(im)

    res = bass_utils.run_bass_kernel_spmd(nc, in_maps, core_ids=list(range(NC)))
    z = np.concatenate([res.results[c]["zT"].T for c in range(NC)], axis=0)
    coords = np.concatenate([res.results[c]["cT"].T for c in range(NC)], axis=0)
    return coords.astype(np.float32), z.astype(np.float32)
